# revision 1
# baseline (speedup 1.0000x reference)
"""Multi-head attention (B=2, N=4096, C=768, H=12, D=64) on 8 TRN2 NeuronCores.

Sharding: tensor-parallel over (batch, head). B*H = 24 pairs -> 3 per core.
Cores 0-3 handle batch 0, cores 4-7 batch 1 (3 consecutive heads each).
Each core computes the QKV projection, attention, and a partial output
projection for its heads, returning a partial y^T [768, 4096]. The host
sums the 4 partials per batch, transposes, and adds the bias.

Host-side prep (layout only, no FLOPs): x is uploaded pre-transposed
(x^T [768, n]) so the contraction dim lands on SBUF partitions without
any on-chip transposes; per-head weight slices are packed so PSUM
partition ranges map straight onto SBUF tiles.

Per-core kernel (Tile framework, bf16 matmul operands, fp32 PSUM accum).
Schedule is paced for the TRN2 PE clock governor: sustained high PE busy-
duty gets clamped to half clock (K=4/8), so Phase A single-buffers its
PSUM drains and Phase C mixes serial/pipelined exp->MM2 emission (8 of 11
k-groups serial) to hold PE duty ~68%. Structure:
  Phase A (per 512-col slice j of x^T):
    - one DMA loads x^T[:, j] as [128, 6, 512]
    - q/k projections head-packed: [q_h0|q_h1] -> qt01 (parts 0:64 / 64:128),
      [k_h0|k_h1] -> kt01, [q_h2|k_h2] -> qt2 + SBUF->SBUF DMA relocation
      of k_h2 from partitions 64:128 down to 0:64.
    - v projection in natural [rows, d] orientation (zero-padded to 256
      cols so the fp32r moving free dim stays >=256), with a ones column
      appended per 65-wide slot to generate softmax denominators.
  Phase C (per q-slice j, head h):
    - MM1: S^T tile [128 k-rows, 512 q] = k^T_chunk.T @ q^T (K=64), 3
      k-chunks per PSUM tensor [128, 1536] (3 banks), double buffered.
    - exp on ScalarE straight from PSUM to SBUF (scores are O(2) by
      construction; max-subtraction unnecessary, mathematically identical).
    - MM2: O^T [65, 512] += v_aug_chunk.T @ P^T accumulated over all 32
      k-chunks in PSUM; row 64 accumulates the softmax denominators.
    - normalize off the PE: DVE reciprocal of row 64, broadcast to 64
      partitions via a DRAM round-trip DMA (partition-stride-0 read),
      one DVE multiply. h0/h1 outputs packed into one 128-partition tile
      (h1 relocated by SBUF->SBUF DMA) so the output projection runs
      K=128+K=64 instead of 3x K=64.
  Phase D (deferred by one head so the PE never waits on the normalize
  chain): y^T chunk [128, 512] accumulated in PSUM, copied, DMA'd out.
"""

import ml_dtypes
import numpy as np

import concourse.bass as bass
import concourse.mybir as mybir
import concourse.tile as tile
from concourse import bacc
from concourse.bass_utils import run_bass_kernel_spmd

F32 = mybir.dt.float32
F32R = mybir.dt.float32r
BF16 = mybir.dt.bfloat16

DIM = 768
NUM_HEADS = 12
HEAD_DIM = 64
SCALE = HEAD_DIM ** -0.5
B = 2
N_FULL = 4096
N_CORES = 8
HEADS_PER_CORE = 3
CC = DIM // 128  # 6 contraction chunks


def build_nc(n=N_FULL, fast_mm=True, pace=True):
    """Build the per-core Bass program. Same program runs SPMD on all
    cores; per-core inputs differ (x^T batch + per-head weight slices)."""
    nj = n // 512      # q slices
    nk = n // 128      # k chunks
    md = BF16 if fast_mm else F32  # dtype for all PE-matmul operands

    nc = bacc.Bacc("TRN2", target_bir_lowering=False, debug=False)

    xt_d = nc.dram_tensor("xt", [DIM, n], md, kind="ExternalInput")
    wqk_d = nc.dram_tensor("wqk", [DIM, 384], md, kind="ExternalInput")
    wv_d = nc.dram_tensor("wv", [DIM, 192], md, kind="ExternalInput")
    wp_d = nc.dram_tensor("wp", [192, DIM], md, kind="ExternalInput")
    yt_d = nc.dram_tensor("yt", [DIM, n], F32, kind="ExternalOutput")

    # MM1 group sizes: 3 k-chunks (3 PSUM banks) per exp instruction.
    groups = [3] * (nk // 3)
    if nk % 3:
        groups.append(nk % 3)

    lp = nc.allow_low_precision(
        reason="float32r is 4-byte fp32; PSUM accumulation stays fp32")
    with lp, tile.TileContext(nc) as tc:
        consts = tc.alloc_tile_pool(name="consts", bufs=1)
        persist = tc.alloc_tile_pool(name="persist", bufs=1)

        # wqk first (needed immediately); wp (Phase C only) goes on the
        # gpsimd queue so it never delays the first x-tile load.
        wqk_sb = consts.tile([128, CC, 384], md, tag="wqk")
        wqk_r = wqk_d[:, :].rearrange("(a p) m -> p a m", p=128)
        nc.sync.dma_start(out=wqk_sb[:, 0:2, :], in_=wqk_r[:, 0:2, :])
        nc.sync.dma_start(out=wqk_sb[:, 2:CC, :], in_=wqk_r[:, 2:CC, :])
        wv_sb = consts.tile([128, CC, 192], md, tag="wv")
        nc.sync.dma_start(out=wv_sb, in_=wv_d[:, :].rearrange("(a p) m -> p a m", p=128))
        wp01_sb = consts.tile([128, DIM], md, tag="wp01")
        nc.gpsimd.dma_start(out=wp01_sb, in_=wp_d[0:128, :])
        wp2_sb = consts.tile([64, DIM], md, tag="wp2")
        nc.gpsimd.dma_start(out=wp2_sb, in_=wp_d[128:192, :])

        # Persistent activations.
        qt01 = persist.tile([128, n], md, tag="qt01")  # parts 0:64 h0, 64:128 h1
        kt01 = persist.tile([128, n], md, tag="kt01")
        qt2 = persist.tile([64, n], md, tag="qt2")
        kt2 = persist.tile([64, n], md, tag="kt2")
        v_aug = [persist.tile([128, nk, 65], md, tag=f"vaug{h}", name=f"vaug{h}")
                 for h in range(HEADS_PER_CORE)]
        for h in range(HEADS_PER_CORE):
            ones_col = v_aug[h][:, :, 64:65]
            if md == F32R:
                ones_col = ones_col.bitcast(F32)
            nc.vector.memset(ones_col, 1.0)

        # Preload the exp table set while ScalarE is otherwise idle so the
        # ~2.7us ACT_TABLE_LOAD doesn't gate the first attention group.
        actwarm = consts.tile([1, 2], F32, tag="actwarm")
        nc.vector.memset(actwarm, 0.0)
        nc.scalar.activation(out=actwarm[0:1, 1:2], in_=actwarm[0:1, 0:1],
                             func=mybir.ActivationFunctionType.Exp)

        # ---- Phase A: QKV projections from pre-transposed x ----
        with (
            tc.tile_pool(name="xtj", bufs=2) as xtj_p,
            tc.tile_pool(name="stage", bufs=2) as stage_p,
            tc.tile_pool(name="qk_ps", bufs=1, space="PSUM") as qk_ps,
            tc.tile_pool(name="v_ps", bufs=1, space="PSUM") as v_ps,
        ):
            for j in range(nj):
                jsl = bass.ts(j, 512)
                xtj = xtj_p.tile([128, CC, 512], md, tag="xtj")
                xt_r = xt_d[:, jsl].rearrange("(a p) m -> p a m", p=128)
                if j == 0:
                    nc.sync.dma_start(out=xtj[:, 0:2, :], in_=xt_r[:, 0:2, :])
                    nc.sync.dma_start(out=xtj[:, 2:CC, :], in_=xt_r[:, 2:CC, :])
                else:
                    nc.sync.dma_start(out=xtj, in_=xt_r)
                # q/k projections: packs [q0|q1], [k0|k1], [q2|k2]
                for pi, colbase in enumerate((0, 128, 256)):
                    ps = qk_ps.tile([128, 512], F32, tag="qk")
                    for cc in range(CC):
                        nc.tensor.matmul(
                            ps,
                            wqk_sb[:, cc, colbase:colbase + 128],
                            xtj[:, cc, :],
                            start=(cc == 0), stop=(cc == CC - 1),
                        )
                    if pi == 0:
                        nc.vector.tensor_copy(out=qt01[:, j * 512:j * 512 + 256], in_=ps[:, 0:256])
                        nc.vector.tensor_copy(out=qt01[:, j * 512 + 256:j * 512 + 512], in_=ps[:, 256:512])
                    elif pi == 1:
                        nc.vector.tensor_copy(out=kt01[:, j * 512:j * 512 + 256], in_=ps[:, 0:256])
                        nc.vector.tensor_copy(out=kt01[:, j * 512 + 256:j * 512 + 512], in_=ps[:, 256:512])
                    else:
                        # q2 -> partitions 0:64 directly; k2 relocated
                        # from partitions 64:128 down to 0:64 via DMA
                        nc.vector.tensor_copy(out=qt2[:, jsl], in_=ps[0:64, :])
                        stage = stage_p.tile([128, 512], md, tag="stage")
                        nc.vector.tensor_copy(out=stage[64:128, :], in_=ps[64:128, :])
                        nc.sync.dma_start(out=kt2[:, jsl], in_=stage[64:128, :])

                # v projection (natural orientation), 3 heads + zero pad
                for rc in range(4):
                    ps = v_ps.tile([128, 192], F32, tag="v")
                    for cc in range(CC):
                        nc.tensor.matmul(
                            ps,
                            xtj[:, cc, bass.ts(rc, 128)],
                            wv_sb[:, cc, :],
                            start=(cc == 0), stop=(cc == CC - 1),
                        )
                    kc = j * 4 + rc
                    for h in range(HEADS_PER_CORE):
                        nc.vector.tensor_copy(
                            out=v_aug[h][:, kc, 0:64], in_=ps[:, bass.ts(h, 64)]
                        )

        # ---- Phase C/D: attention + output projection ----
        with (
            tc.tile_pool(name="s_ps", bufs=2, space="PSUM") as s_ps,
            tc.tile_pool(name="o_ps", bufs=2, space="PSUM") as o_ps,
            tc.tile_pool(name="dscr", bufs=6, space="DRAM") as dscr_p,
            tc.tile_pool(name="ptp", bufs=6) as ptp,
            tc.tile_pool(name="otp", bufs=4) as otp,
            tc.tile_pool(name="obp", bufs=3) as obp,
            tc.tile_pool(name="rsbp", bufs=4) as rsbp,
            tc.tile_pool(name="ytp", bufs=4) as ytp,
        ):
            st = {"pending": None, "ots": [], "prev": None, "credit": 0}
            out_q = []          # deferred outproj cc-pieces
            # The HW clamps the PE to half clock when its busy duty per
            # ~3.4us window stays high (dummy matmul fillers made this WORSE,
            # measured: 993us vs 659us). Pace the PE by mixing emission
            # styles per k-group: 'serial' groups put MM2(g) directly behind
            # exp(g) in the PE queue (PE idles ~900ns there, like the warm-
            # stable baseline), 'pipelined' groups run MM2 one group behind
            # (no idle). SERIAL_EVERY tunes the duty.
            # serial on 8 of 11 groups -> PE duty ~68% in Phase C
            def is_serial(gi):
                return gi % 4 != 0

            def emit_op_piece(pj, cc, ot01, ot2):
                pjsl = bass.ts(pj, 512)
                yps = o_ps.tile([128, 512], F32, tag="o", name="yps")
                nc.tensor.matmul(
                    yps, wp01_sb[:, bass.ts(cc, 128)], ot01,
                    start=True, stop=False,
                )
                nc.tensor.matmul(
                    yps, wp2_sb[:, bass.ts(cc, 128)], ot2,
                    start=False, stop=True,
                )
                yst = ytp.tile([128, 512], F32, tag="yt")
                nc.vector.tensor_copy(out=yst, in_=yps)
                nc.gpsimd.dma_start(out=yt_d[bass.ts(cc, 128), pjsl], in_=yst)

            def emit_fill(o_t):
                """Per-group point for real deferred PE work (outproj)."""
                if out_q:
                    emit_op_piece(*out_q.pop(0))

            def finish_head(o_t, j, h):
                # Drain o_t to SBUF immediately so the PSUM buffer recycles
                # fast; normalize entirely off the PE from the SBUF copy.
                ob = obp.tile([128, 512], F32, tag="ob")
                nc.vector.tensor_copy(out=ob[0:65, :], in_=o_t[0:65, :])
                scr = dscr_p.tile([512], F32, tag="scr")
                nc.sync.dma_start(out=scr, in_=ob[64:65, :])
                # reshape to [128,4] so the 8-cycle/elem reciprocal runs on
                # 128 lanes x 4 elems instead of 1 lane x 512
                r4 = rsbp.tile([128, 4], F32, tag="r4")
                nc.sync.dma_start(out=r4, in_=scr.rearrange("(p f) -> p f", p=128))
                r4i = rsbp.tile([128, 4], F32, tag="r4i")
                nc.vector.reciprocal(out=r4i, in_=r4)
                scr2 = dscr_p.tile([512], F32, tag="scr2")
                nc.sync.dma_start(out=scr2, in_=r4i)
                bcs = rsbp.tile([64, 512], F32, tag="bcs")
                scr_b = bass.AP(tensor=scr2.tensor, offset=scr2.offset,
                                ap=[[0, 64]] + list(scr2.ap))
                nc.sync.dma_start(out=bcs, in_=scr_b)
                ots = st["ots"]
                if h == 0:
                    ot01 = otp.tile([128, 512], md, tag="ot01", name="ot01")
                    nc.vector.tensor_mul(ot01[0:64, :], ob[0:64, :], bcs)
                    ots.append(ot01)
                    if st["pending"] is not None:
                        pj, pots = st["pending"]
                        for cc in range(CC):
                            out_q.append((pj, cc, pots[0], pots[1]))
                        st["pending"] = None
                elif h == 1:
                    ot1 = otp.tile([64, 512], md, tag="ot1", name="ot1")
                    nc.vector.tensor_mul(ot1, ob[0:64, :], bcs)
                    nc.sync.dma_start(out=ots[0][64:128, :], in_=ot1)
                else:
                    ot2 = otp.tile([64, 512], md, tag="ot2", name="ot2")
                    nc.vector.tensor_mul(ot2, ob[0:64, :], bcs)
                    ots.append(ot2)
                    st["pending"] = (j, list(ots))
                    ots.clear()

            def emit_mm2(u):
                o_t, ptt, ks, gsize, is_last, j, h = u
                for t in range(gsize):
                    kc = ks + t
                    nc.tensor.matmul(
                        o_t[0:65, :], v_aug[h][:, kc, :],
                        ptt[:, bass.ts(t, 512)],
                        start=(kc == 0), stop=(kc == nk - 1),
                    )
                if is_last:
                    finish_head(o_t, j, h)

            for j in range(nj):
                jsl = bass.ts(j, 512)
                for h in range(HEADS_PER_CORE):
                    if h == 0:
                        q_sl, kt_t, kbase = qt01[0:64, jsl], kt01, 0
                    elif h == 1:
                        q_sl, kt_t, kbase = qt01[64:128, jsl], kt01, 64
                    else:
                        q_sl, kt_t, kbase = qt2[:, jsl], kt2, 0
                    o_t = o_ps.tile([128, 512], F32, tag="o")
                    ks = 0
                    for gi, gsize in enumerate(groups):
                        sp = s_ps.tile([128, 1536], F32, tag="s")
                        for t in range(gsize):
                            kc = ks + t
                            lhs = kt_t[kbase:kbase + 64, bass.ts(kc, 128)]
                            nc.tensor.matmul(
                                sp[:, bass.ts(t, 512)], lhs, q_sl,
                                start=True, stop=True,
                            )
                        ptt = ptp.tile([128, 1536], md, tag="pt")
                        nc.scalar.activation(
                            out=ptt[:, 0:gsize * 512], in_=sp[:, 0:gsize * 512],
                            func=mybir.ActivationFunctionType.Exp,
                        )
                        prev = st["prev"]
                        emit_fill(prev[0] if prev is not None else o_t)
                        if prev is not None:
                            emit_mm2(prev)
                        cur = (o_t, ptt, ks, gsize,
                               gi == len(groups) - 1, j, h)
                        if is_serial(gi):
                            emit_mm2(cur)      # serial: PE waits exp here
                            st["prev"] = None
                        else:
                            st["prev"] = cur   # pipelined: MM2 next round
                        ks += gsize
            if st["prev"] is not None:
                emit_mm2(st["prev"])
            while out_q:
                emit_op_piece(*out_q.pop(0))
            pj, pots = st["pending"]
            pjsl = bass.ts(pj, 512)
            for cc in range(CC):
                yps = o_ps.tile([128, 512], F32, tag="o", name="yps")
                nc.tensor.matmul(yps, wp01_sb[:, bass.ts(cc, 128)], pots[0],
                                 start=True, stop=False)
                nc.tensor.matmul(yps, wp2_sb[:, bass.ts(cc, 128)], pots[1],
                                 start=False, stop=True)
                yst = ytp.tile([128, 512], F32, tag="yt")
                # alternate drain engines and store queues so the tail's six
                # stores land in parallel instead of serially
                if cc % 2 == 0:
                    nc.vector.tensor_copy(out=yst, in_=yps)
                    nc.sync.dma_start(out=yt_d[bass.ts(cc, 128), pjsl], in_=yst)
                else:
                    nc.scalar.copy(out=yst, in_=yps)
                    nc.gpsimd.dma_start(out=yt_d[bass.ts(cc, 128), pjsl], in_=yst)

        persist.release()
        consts.release()

    nc.compile()
    return nc


def make_core_inputs(x_b, w_qkv, w_proj, hbase, fast_mm=True):
    """Per-core input arrays for heads [hbase, hbase+3) of batch x_b."""
    C = DIM
    wq = [w_qkv[(hbase + h) * 64:(hbase + h + 1) * 64, :] * SCALE for h in range(3)]
    wk = [w_qkv[C + (hbase + h) * 64:C + (hbase + h + 1) * 64, :] for h in range(3)]
    wv = [w_qkv[2 * C + (hbase + h) * 64:2 * C + (hbase + h + 1) * 64, :] for h in range(3)]

    wqk = np.zeros((C, 384), np.float32)
    wqk[:, 0:64] = wq[0].T
    wqk[:, 64:128] = wq[1].T
    wqk[:, 128:192] = wk[0].T
    wqk[:, 192:256] = wk[1].T
    wqk[:, 256:320] = wq[2].T
    wqk[:, 320:384] = wk[2].T

    wv_p = np.zeros((C, 192), np.float32)
    for h in range(3):
        wv_p[:, h * 64:(h + 1) * 64] = wv[h].T

    wp = np.zeros((192, C), np.float32)
    for h in range(3):
        wp[h * 64:(h + 1) * 64, :] = w_proj[:, (hbase + h) * 64:(hbase + h + 1) * 64].T

    dt = ml_dtypes.bfloat16 if fast_mm else np.float32
    return {
        "xt": np.ascontiguousarray(x_b.T).astype(dt),
        "wqk": wqk.astype(dt),
        "wv": wv_p.astype(dt),
        "wp": wp.astype(dt),
    }


_NC_CACHE = {}


def get_nc(n=N_FULL, fast_mm=True, pace=True):
    key = (n, fast_mm, pace)
    if key not in _NC_CACHE:
        _NC_CACHE[key] = build_nc(n, fast_mm, pace)
    return _NC_CACHE[key]


def kernel(x, w_qkv, w_proj, b_proj, _trace=False):
    x = np.asarray(x, np.float32)
    w_qkv = np.asarray(w_qkv, np.float32)
    w_proj = np.asarray(w_proj, np.float32)
    b_proj = np.asarray(b_proj, np.float32)

    nc = get_nc(N_FULL, True)
    in_maps = []
    for c in range(N_CORES):
        b = c // 4
        hbase = (c % 4) * HEADS_PER_CORE
        in_maps.append(make_core_inputs(x[b], w_qkv, w_proj, hbase))

    res = run_bass_kernel_spmd(nc, in_maps, core_ids=list(range(N_CORES)),
                               trace=_trace)
    y = np.empty((B, N_FULL, DIM), np.float32)
    for b in range(B):
        acc = res.results[4 * b]["yt"].astype(np.float32)
        for c in range(4 * b + 1, 4 * b + 4):
            acc = acc + res.results[c]["yt"]
        y[b] = acc.T + b_proj[None, :]
    if _trace:
        return y, res
    return y

